# revision 2
# baseline (speedup 1.0000x reference)
"""MultiHeadSelfAttention TRN2 kernel — head-parallel, fp8-DoubleRow ZT path.

Reference semantics (softmax over the QUERY axis):
    Q = x @ Wq[h].T + bq[h]; K likewise; V likewise
    P[s,t] = exp((Q[s]·K[t])/sqrt(D) - shift)        (shift cancels in a)
    denom[t] = sum_s P[s,t]
    Z[s] = sum_t (P[s,t]/denom[t]) V[t]
    out = concat_heads(Z) @ Wo.T + bo

v2 changes vs v1:
  * P written as fp8 (e5m2) by ACT exp; V'' = V * (VS/denom) as fp8 (e4m3).
  * ZT = sum_t V''^T P uses fp8 DoubleRow matmuls (0.55 c/row measured on HW
    vs 1.0 f32r) accumulated directly in PSUM across all 16 t-blocks
    (kills the v1 SBUF Z-accumulation DVE passes).
  * out-proj bias moved to ACT; Z psum->SBUF scale-copy on DVE.
  * batch-level SW pipeline: ZT(b-1)+out(b-1) interleave into scores(b).

Scales: scores f32r exact. P = exp(sc - SHIFT). VS ~ typical denom so V'' ~ O(1).
zt = (1/VS) * psumZ, so out-proj sees true Z.
"""

import numpy as np

import concourse.bass as bass
import concourse.mybir as mybir
import concourse.tile as tile
from concourse import bacc
from concourse.bass_utils import run_bass_kernel_spmd

B, S, D, H = 4, 2048, 256, 8
N_CORES = 8
P = 128          # partitions
NDB = D // P     # 2 d-blocks
NTB = S // P     # 16 t-blocks
SC = 512         # matmul moving chunk
NSC = S // SC    # 4
SH = 1024        # ACT/psum tile width
NSH = S // SH    # 2
VG = 4           # V proj t-blocks per psum group

SHIFT = 6.0      # exp bias shift: P=exp(sc-SHIFT) fits e4m3 (max |sc|~9)
VS = 16.0        # V'' scale ~ typical denom at shift 6; 1/VS folds into woT

f32 = mybir.dt.float32
f32r = mybir.dt.float32r
f8e4 = mybir.dt.float8e4
f8e5 = mybir.dt.float8e5
DR = mybir.MatmulPerfMode.DoubleRow
EXP = mybir.ActivationFunctionType.Exp
IDENT = mybir.ActivationFunctionType.Identity


def _build():
    nc = bacc.Bacc(target_bir_lowering=False)

    xT = nc.dram_tensor("xT", [B, D, S], f32, kind="ExternalInput")
    wqT = nc.dram_tensor("wqT", [D, D], f32, kind="ExternalInput")  # (Wq/sqrt(D)).T
    wkT = nc.dram_tensor("wkT", [D, D], f32, kind="ExternalInput")
    wvT = nc.dram_tensor("wvT", [D, D], f32, kind="ExternalInput")
    woT = nc.dram_tensor("woT", [D, D], f32, kind="ExternalInput")
    bqc = nc.dram_tensor("bqc", [D, 1], f32, kind="ExternalInput")
    bkc = nc.dram_tensor("bkc", [D, 1], f32, kind="ExternalInput")
    bvb = nc.dram_tensor("bvb", [P, VG * D], f32, kind="ExternalInput")
    boc = nc.dram_tensor("boc", [D, 1], f32, kind="ExternalInput")
    outT = nc.dram_tensor("outT", [B, D, S], f32, kind="ExternalOutput")

    with tile.TileContext(nc) as tc:
        with (
            tc.tile_pool(name="const", bufs=1) as cpool,
            tc.tile_pool(name="big", bufs=1) as xpool,
            tc.tile_pool(name="pt", bufs=2) as ppool,
            tc.tile_pool(name="small", bufs=2) as spool,
            tc.tile_pool(name="outp", bufs=2) as opool,
            tc.tile_pool(name="ps_a", bufs=3, space="PSUM") as psa,
            tc.tile_pool(name="ps_z", bufs=2, space="PSUM") as psz,
        ):
            # ---- constants ----
            wq_t = cpool.tile([P, NDB, D], f32r, tag="wq")
            wk_t = cpool.tile([P, NDB, D], f32r, tag="wk")
            wv_t = cpool.tile([P, NDB, D], f32r, tag="wv")
            wo_t = cpool.tile([P, NDB, D], f32r, tag="wo")
            bq_t = cpool.tile([P, NDB, 1], f32, tag="bq")
            bk_t = cpool.tile([P, NDB, 1], f32, tag="bk")
            bo_t = cpool.tile([P, NDB, 1], f32, tag="bo")
            bvb_t = cpool.tile([P, VG * D], f32, tag="bvb")
            # wq + the q/k biases on the scalar queue (gate the first PE/ACT work);
            # everything else behind them on the gpsimd queue
            nc.scalar.dma_start(
                out=wq_t[:], in_=wqT.rearrange("(n p) e -> p n e", p=P).bitcast(f32r)
            )
            for b_t, b_d in ((bq_t, bqc), (bk_t, bkc)):
                nc.scalar.dma_start(
                    out=b_t[:], in_=b_d.rearrange("(n p) o -> p n o", p=P)
                )
            # wo last: not needed until the first out-proj (~60us in)
            for w_t, w_d in ((wk_t, wkT), (wv_t, wvT)):
                nc.gpsimd.dma_start(
                    out=w_t[:], in_=w_d.rearrange("(n p) e -> p n e", p=P).bitcast(f32r)
                )
            nc.gpsimd.dma_start(out=bvb_t[:], in_=bvb[:])
            nc.gpsimd.dma_start(
                out=wo_t[:], in_=woT.rearrange("(n p) e -> p n e", p=P).bitcast(f32r)
            )
            nc.gpsimd.dma_start(
                out=bo_t[:], in_=boc.rearrange("(n p) o -> p n o", p=P)
            )
            shift_t = cpool.tile([P, 1], f32, tag="shiftc")
            nc.vector.memset(shift_t[:], -SHIFT)

            # ---- persistent tiles (single-generation; Tile tracks hazards) ----
            def batch_tiles():
                return dict(
                    pt=ppool.tile([P, NTB, S], f8e4, tag="pt", name="pt"),
                    vp=ppool.tile([P, NTB, D], f8e4, tag="vp", name="vp"),
                    vp2=ppool.tile([P, NTB, D], f8e4, tag="vp2", name="vp2"),
                    dnp=spool.tile([P, NTB, NSH], f32, tag="dnp", name="dnp"),
                    rc=spool.tile([P, NTB], f32, tag="rc", name="rc"),
                )

            # ======== emit helpers (b = batch idx) ========
            def emit_xload(b, xt, chunks=2):
                xT_r = xT[b].rearrange("(n p) s -> p n s", p=P).bitcast(f32r)
                w = S // chunks
                for c in range(chunks):
                    nc.sync.dma_start(
                        out=xt[:, :, bass.ds(c * w, w)], in_=xT_r[:, :, bass.ds(c * w, w)]
                    )

            def emit_qk_group(b, xt, qt, kt, gi):
                """One Q/K projection psum group: gi in 0..7 -> (q/k, eb, sh)."""
                dst, w, bias = ((qt, wq_t, bq_t), (kt, wk_t, bk_t))[gi // 4]
                eb, sh = (gi // 2) % 2, gi % 2
                ps = psa.tile([P, SH], f32, tag="acc")
                for sc in range(SH // SC):
                    ssl = bass.ds(sh * SH + sc * SC, SC)
                    psl = bass.ts(sc, SC)
                    for db in range(NDB):
                        nc.tensor.matmul(
                            ps[:, psl],
                            w[:, db, bass.ts(eb, P)],
                            xt[:, db, ssl],
                            start=(db == 0),
                            stop=(db == NDB - 1),
                        )
                # bias-add on ACT for b>0 (idle during proj); DVE for b==0
                # (batch 0's ACT is fully loaded by its own exps)
                if b == 0:
                    nc.vector.tensor_scalar_add(
                        dst[:, eb, bass.ts(sh, SH)], ps[:], bias[:, eb, :]
                    )
                else:
                    nc.scalar.activation(
                        dst[:, eb, bass.ts(sh, SH)], ps[:], IDENT, bias=bias[:, eb, :]
                    )

            def emit_v_group(b, xt, v_all, vg):
                psv = psa.tile([P, VG * D], f32, tag="acc")
                for k in range(VG):
                    tb = vg * VG + k
                    for db in range(NDB):
                        nc.tensor.matmul(
                            psv[:, bass.ts(k, D)],
                            xt[:, db, bass.ts(tb, P)],
                            wv_t[:, db, :],
                            start=(db == 0),
                            stop=(db == NDB - 1),
                        )
                nc.vector.tensor_add(
                    v_all[:, bass.ds(vg * VG, VG), :],
                    psv[:].rearrange("p (g e) -> p g e", g=VG),
                    bvb_t[:].rearrange("p (g e) -> p g e", g=VG),
                )

            def emit_scores_tb(b, t, qt, kt, v_all, tb):
                """scores + exp->fp8 P for one t-block; after the 4th tb in a
                group of 4, emit the denom/reciprocal/V'' DVE ops for the group."""
                for sh in range(NSH):
                    pssc = psa.tile([P, SH], f32, tag="acc")
                    for sc in range(SH // SC):
                        ssl = bass.ds(sh * SH + sc * SC, SC)
                        psl = bass.ts(sc, SC)
                        for eb in range(NDB):
                            nc.tensor.matmul(
                                pssc[:, psl],
                                kt[:, eb, bass.ts(tb, P)],
                                qt[:, eb, ssl],
                                start=(eb == 0),
                                stop=(eb == NDB - 1),
                            )
                    nc.scalar.activation(
                        t["pt"][:, tb, bass.ts(sh, SH)],
                        pssc[:],
                        EXP,
                        bias=shift_t[:],
                        accum_out=t["dnp"][:, tb, sh : sh + 1],
                    )
                if tb % 4 == 3:
                    g0 = tb - 3
                    dn = spool.tile([P, 4], f32, tag="dn")
                    nc.vector.tensor_add(
                        dn[:], t["dnp"][:, bass.ds(g0, 4), 0], t["dnp"][:, bass.ds(g0, 4), 1]
                    )
                    nc.vector.reciprocal(t["rc"][:, bass.ds(g0, 4)], dn[:])
                    for j in range(g0, g0 + 4):
                        # V'' split: t16 = 16*V'', vp = e4m3(V''), vp2 = e4m3(16*(V''-vp))
                        t16 = spool.tile([P, D], f32, tag="t16")
                        nc.vector.tensor_scalar(
                            t16[:],
                            v_all[:, j, :],
                            t["rc"][:, j : j + 1],
                            16.0 * VS,
                            mybir.AluOpType.mult,
                            mybir.AluOpType.mult,
                        )
                        nc.vector.tensor_scalar_mul(t["vp"][:, j, :], t16[:], 1.0 / 16.0)
                        # NOTE: scalar_tensor_tensor with fp8 OUT is broken on HW;
                        # use mul->f32 temp + sub->fp8 instead
                        u16 = spool.tile([P, D], f32, tag="u16")
                        nc.vector.tensor_scalar_mul(u16[:], t["vp"][:, j, :], 16.0)
                        nc.vector.tensor_sub(t["vp2"][:, j, :], t16[:], u16[:])

            def emit_zt_chunk(b, t, zt, ci, lo=False, jr=(0, NTB // 2), pszt=None):
                """One ZT psum sub-chunk (hi: V1 or lo: V2 sum) for (sh, eh, sq).
                jr limits the pair range (split emission keeps the psum group
                open across calls); the zt combine fires when the group closes."""
                sh, eh, sq2 = ci // 4, (ci // 2) % 2, ci % 2
                sq = sh * 2 + sq2
                vsrc = t["vp2"] if lo else t["vp"]
                if pszt is None:
                    pszt = psz.tile([P, SC], f32, tag="z")
                ssl = bass.ts(sq, SC)
                for j in range(*jr):
                    nc.tensor.matmul(
                        pszt[:],
                        vsrc[:, bass.ds(2 * j, 2), bass.ts(eh, P)],
                        t["pt"][:, bass.ds(2 * j, 2), ssl],
                        start=(j == 0),
                        stop=(j == NTB // 2 - 1),
                        perf_mode=DR,
                    )
                if jr[1] < NTB // 2:
                    return pszt
                zsl = zt[:, eh, ssl]
                if not lo:
                    nc.vector.tensor_copy(zsl, pszt[:])
                else:
                    nc.vector.scalar_tensor_tensor(
                        zsl, pszt[:], 1.0 / 16.0, zsl,
                        mybir.AluOpType.mult, mybir.AluOpType.add,
                    )
                return None

            def emit_out_group(b, zt, gi, fan=2):
                """Out-proj psum group (ob, sh): 4 matmuls + DVE bias + DMA."""
                ob, sh = gi // NSH, gi % NSH
                pso = psa.tile([P, SH], f32, tag="acc")
                for sc in range(SH // SC):
                    ssl = bass.ds(sh * SH + sc * SC, SC)
                    psl = bass.ts(sc, SC)
                    for eh in range(NDB):
                        nc.tensor.matmul(
                            pso[:, psl],
                            wo_t[:, eh, bass.ts(ob, P)],
                            zt[:, eh, ssl],
                            start=(eh == 0),
                            stop=(eh == NDB - 1),
                        )
                osb = opool.tile([P, SH], f32, tag="osb")
                if fan == 2:
                    nc.vector.tensor_scalar_add(osb[:], pso[:], bo_t[:, ob, :])
                    dma_eng = nc.sync if gi % 2 == 0 else nc.gpsimd
                    dma_eng.dma_start(
                        out=outT[b, bass.ts(ob, P), bass.ts(sh, SH)], in_=osb[:]
                    )
                else:
                    # final batch: per-512 bias+DMA on sync/scalar queues only
                    # (keep the gpsimd queue empty for a cheap final drain)
                    engs = (nc.sync, nc.scalar)
                    for q in range(2):
                        qsl = bass.ts(q, SH // 2)
                        nc.vector.tensor_scalar_add(
                            osb[:, qsl], pso[:, qsl], bo_t[:, ob, :]
                        )
                        hsl = bass.ds(sh * SH + q * (SH // 2), SH // 2)
                        engs[q].dma_start(
                            out=outT[b, bass.ts(ob, P), hsl], in_=osb[:, qsl]
                        )

            # ======== schedule ========
            xt = xpool.tile([P, NDB, S], f32r, tag="xt")
            qt = xpool.tile([P, NDB, S], f32r, tag="qt")
            kt = xpool.tile([P, NDB, S], f32r, tag="kt")
            v_all = xpool.tile([P, NTB, D], f32, tag="v")
            zt = xpool.tile([P, NDB, S], f32r, tag="zt")

            # interleave item lists: "c<i>h"/"c<i>l" = ZT(b-1) sub-chunk hi/lo,
            # "g<i>" = out(b-1) group
            PROJ_SLOTS = {2: "c0h", 4: "c0l", 6: "c1h", 8: "c1l",
                          10: "c2h", 12: "c2l"}
            SCORE_SLOTS = {2: "c3h", 3: "c3l", 4: "c4h", 5: "g0",
                           6: "c4l", 7: "c5h", 8: "g2", 9: "c5l",
                           10: "c6h", 11: "c6l", 12: "c7h", 13: "c7l",
                           14: "g1", 15: "g3"}

            def emit_item(item, pb, pt_, fan=2):
                if item[0] == "c":
                    emit_zt_chunk(pb, pt_, zt, int(item[1]), lo=item[2] == "l")
                else:
                    emit_out_group(pb, zt, int(item[1]), fan=fan)

            prev = None   # (b-1, tiles)
            for b in range(B):
                cur = batch_tiles()
                if b == 0:
                    emit_xload(b, xt, chunks=4)
                # projections with ZT(b-1) sub-chunks interleaved
                pg = 0
                for gi in range(8):
                    emit_qk_group(b, xt, qt, kt, gi)
                    pg += 1
                    if prev is not None and pg in PROJ_SLOTS:
                        emit_item(PROJ_SLOTS[pg], *prev)
                if prev is not None:
                    for vg in range(NTB // VG):
                        emit_v_group(b, xt, v_all, vg)
                        pg += 1
                        if pg in PROJ_SLOTS:
                            emit_item(PROJ_SLOTS[pg], *prev)
                if prev is not None and b + 1 < B:
                    emit_xload(b + 1, xt)
                for tb in range(NTB):
                    emit_scores_tb(b, cur, qt, kt, v_all, tb)
                    if prev is not None and tb in SCORE_SLOTS:
                        emit_item(SCORE_SLOTS[tb], *prev)
                    elif prev is None and tb % 4 == 0:
                        # batch 0: V-proj groups spread across the phase
                        emit_v_group(b, xt, v_all, tb // 4)
                        if tb // 4 == NTB // VG - 1:
                            # xt(1) load only after the last V-proj read of xt(0)
                            emit_xload(b + 1, xt)
                prev = (b, cur)
            # tail: last batch's ZT + out. The first two hi chunks start with
            # pairs 0..5 (need only vp[0..11], ready before the final exps),
            # closing with pairs 6,7 once the last vp group lands.
            pb, pt_ = prev
            h6 = NTB // 2 - 2
            ps_c0 = emit_zt_chunk(pb, pt_, zt, 0, jr=(0, h6))
            ps_c1 = emit_zt_chunk(pb, pt_, zt, 1, jr=(0, h6))
            emit_zt_chunk(pb, pt_, zt, 0, jr=(h6, NTB // 2), pszt=ps_c0)
            emit_zt_chunk(pb, pt_, zt, 1, jr=(h6, NTB // 2), pszt=ps_c1)
            for item in ("c2h", "c3h", "c0l", "c1l", "c2l", "c3l",
                         "g0", "c4h", "c5h", "g2", "c6h", "c7h", "c4l", "c5l",
                         "c6l", "c7l", "g1", "g3"):
                emit_item(item, pb, pt_, fan=4)

    nc.compile()
    return nc


_NC = None


def _get_nc():
    global _NC
    if _NC is None:
        _NC = _build()
    return _NC


def _make_in_maps(x, Wq, bq, Wk, bk, Wv, bv, Wo, bo):
    x = np.asarray(x, np.float32)
    scale = np.float32(1.0 / np.sqrt(D))
    xT = np.ascontiguousarray(x.transpose(0, 2, 1))
    in_maps = []
    for h in range(H):
        bvh = np.asarray(bv, np.float32)[h]
        m = {
            "xT": xT,
            "wqT": np.ascontiguousarray(np.asarray(Wq, np.float32)[h].T) * scale,
            "wkT": np.ascontiguousarray(np.asarray(Wk, np.float32)[h].T),
            "wvT": np.ascontiguousarray(np.asarray(Wv, np.float32)[h].T),
            "woT": np.ascontiguousarray(np.asarray(Wo, np.float32)[:, h * D : (h + 1) * D].T)
            * (1.0 / VS),
            "bqc": (np.asarray(bq, np.float32)[h] * scale).reshape(D, 1),
            "bkc": np.asarray(bk, np.float32)[h].reshape(D, 1),
            "bvb": np.ascontiguousarray(
                np.broadcast_to(np.tile(bvh, VG), (P, VG * D))
            ),
            "boc": (
                np.asarray(bo, np.float32) if h == 0 else np.zeros(D, np.float32)
            ).reshape(D, 1),
        }
        in_maps.append({k: np.ascontiguousarray(v, np.float32) for k, v in m.items()})
    return in_maps


def kernel(x, Wq, bq, Wk, bk, Wv, bv, Wo, bo, _trace=False, _trace_kwargs=None):
    in_maps = _make_in_maps(x, Wq, bq, Wk, bk, Wv, bv, Wo, bo)
    nc = _get_nc()
    kw = {}
    if _trace:
        kw = dict(trace=True, **(_trace_kwargs or {}))
    br = run_bass_kernel_spmd(nc, in_maps, core_ids=list(range(N_CORES)), **kw)
    acc = np.zeros((B, D, S), np.float32)
    for r in br.results:
        acc += r["outT"]
    out = np.ascontiguousarray(acc.transpose(0, 2, 1))
    if _trace:
        kernel.last_results = br
    return out


# revision 3
# speedup vs baseline: 1.0107x; 1.0107x over previous
"""MultiHeadSelfAttention TRN2 kernel — head-parallel, fp8-DoubleRow ZT path.

Reference semantics (softmax over the QUERY axis):
    Q = x @ Wq[h].T + bq[h]; K likewise; V likewise
    P[s,t] = exp((Q[s]·K[t])/sqrt(D) - shift)        (shift cancels in a)
    denom[t] = sum_s P[s,t]
    Z[s] = sum_t (P[s,t]/denom[t]) V[t]
    out = concat_heads(Z) @ Wo.T + bo

v2 changes vs v1:
  * P written as fp8 e4m3 by ACT exp (shift 6 avoids the 240 clip);
    V'' = V * (VS/denom) split hi/lo into two e4m3 tensors (lo = 16x residual)
    to survive the 2e-2 gate with pre-quantization (ACT accum) denominators.
  * ZT = sum_t V''^T P uses fp8 DoubleRow matmuls (0.55 c/row measured on HW
    vs 1.0 f32r) accumulated directly in PSUM across all 16 t-blocks
    (kills the v1 SBUF Z-accumulation DVE passes).
  * out-proj bias moved to ACT; Z psum->SBUF scale-copy on DVE.
  * batch-level SW pipeline: ZT(b-1)+out(b-1) interleave into scores(b).

Scales: scores f32r exact. P = exp(sc - SHIFT). VS ~ typical denom so V'' ~ O(1).
zt = (1/VS) * psumZ, so out-proj sees true Z.
"""

import numpy as np

import concourse.bass as bass
import concourse.mybir as mybir
import concourse.tile as tile
from concourse import bacc
from concourse.bass_utils import run_bass_kernel_spmd

B, S, D, H = 4, 2048, 256, 8
N_CORES = 8
P = 128          # partitions
NDB = D // P     # 2 d-blocks
NTB = S // P     # 16 t-blocks
SC = 512         # matmul moving chunk
NSC = S // SC    # 4
SH = 1024        # ACT/psum tile width
NSH = S // SH    # 2
VG = 4           # V proj t-blocks per psum group

SHIFT = 6.0      # exp bias shift: P=exp(sc-SHIFT) fits e4m3 (max |sc|~9)
VS = 16.0        # V'' scale ~ typical denom at shift 6; 1/VS folds into woT

f32 = mybir.dt.float32
f32r = mybir.dt.float32r
f8e4 = mybir.dt.float8e4
f8e5 = mybir.dt.float8e5
DR = mybir.MatmulPerfMode.DoubleRow
EXP = mybir.ActivationFunctionType.Exp
IDENT = mybir.ActivationFunctionType.Identity


def _build():
    nc = bacc.Bacc(target_bir_lowering=False)

    xT = nc.dram_tensor("xT", [B, D, S], f32, kind="ExternalInput")
    wqT = nc.dram_tensor("wqT", [D, D], f32, kind="ExternalInput")  # (Wq/sqrt(D)).T
    wkT = nc.dram_tensor("wkT", [D, D], f32, kind="ExternalInput")
    wvT = nc.dram_tensor("wvT", [D, D], f32, kind="ExternalInput")
    woT = nc.dram_tensor("woT", [D, D], f32, kind="ExternalInput")
    bqc = nc.dram_tensor("bqc", [D, 1], f32, kind="ExternalInput")
    bkc = nc.dram_tensor("bkc", [D, 1], f32, kind="ExternalInput")
    bvb = nc.dram_tensor("bvb", [P, VG * D], f32, kind="ExternalInput")
    boc = nc.dram_tensor("boc", [D, 1], f32, kind="ExternalInput")
    outT = nc.dram_tensor("outT", [B, D, S], f32, kind="ExternalOutput")

    with tile.TileContext(nc) as tc:
        with (
            tc.tile_pool(name="const", bufs=1) as cpool,
            tc.tile_pool(name="big", bufs=1) as xpool,
            tc.tile_pool(name="pt", bufs=2) as ppool,
            tc.tile_pool(name="small", bufs=2) as spool,
            tc.tile_pool(name="outp", bufs=2) as opool,
            tc.tile_pool(name="ps_a", bufs=3, space="PSUM") as psa,
            tc.tile_pool(name="ps_z", bufs=2, space="PSUM") as psz,
        ):
            # ---- constants ----
            wq_t = cpool.tile([P, NDB, D], f32r, tag="wq")
            wk_t = cpool.tile([P, NDB, D], f32r, tag="wk")
            wv_t = cpool.tile([P, NDB, D], f32r, tag="wv")
            wo_t = cpool.tile([P, NDB, D], f32r, tag="wo")
            bq_t = cpool.tile([P, NDB, 1], f32, tag="bq")
            bk_t = cpool.tile([P, NDB, 1], f32, tag="bk")
            bo_t = cpool.tile([P, NDB, 1], f32, tag="bo")
            bvb_t = cpool.tile([P, VG * D], f32, tag="bvb")
            # wq + the q/k biases on the scalar queue (gate the first PE/ACT work);
            # everything else behind them on the gpsimd queue
            nc.scalar.dma_start(
                out=wq_t[:], in_=wqT.rearrange("(n p) e -> p n e", p=P).bitcast(f32r)
            )
            for b_t, b_d in ((bq_t, bqc), (bk_t, bkc)):
                nc.scalar.dma_start(
                    out=b_t[:], in_=b_d.rearrange("(n p) o -> p n o", p=P)
                )
            # wo last: not needed until the first out-proj (~60us in)
            for w_t, w_d in ((wk_t, wkT), (wv_t, wvT)):
                nc.gpsimd.dma_start(
                    out=w_t[:], in_=w_d.rearrange("(n p) e -> p n e", p=P).bitcast(f32r)
                )
            nc.gpsimd.dma_start(out=bvb_t[:], in_=bvb[:])
            nc.gpsimd.dma_start(
                out=wo_t[:], in_=woT.rearrange("(n p) e -> p n e", p=P).bitcast(f32r)
            )
            nc.gpsimd.dma_start(
                out=bo_t[:], in_=boc.rearrange("(n p) o -> p n o", p=P)
            )
            shift_t = cpool.tile([P, 1], f32, tag="shiftc")
            nc.vector.memset(shift_t[:], -SHIFT)

            # ---- persistent tiles (single-generation; Tile tracks hazards) ----
            def batch_tiles():
                return dict(
                    pt=ppool.tile([P, NTB, S], f8e4, tag="pt", name="pt"),
                    vp=ppool.tile([P, NTB, D], f8e4, tag="vp", name="vp"),
                    vp2=ppool.tile([P, NTB, D], f8e4, tag="vp2", name="vp2"),
                    dnp=spool.tile([P, NTB, NSH], f32, tag="dnp", name="dnp"),
                    rc=spool.tile([P, NTB], f32, tag="rc", name="rc"),
                )

            # ======== emit helpers (b = batch idx) ========
            def emit_xload(b, xt, chunks=2):
                xT_r = xT[b].rearrange("(n p) s -> p n s", p=P).bitcast(f32r)
                w = S // chunks
                for c in range(chunks):
                    nc.sync.dma_start(
                        out=xt[:, :, bass.ds(c * w, w)], in_=xT_r[:, :, bass.ds(c * w, w)]
                    )

            def emit_qk_group(b, xt, qt, kt, gi):
                """One Q/K projection psum group: gi in 0..7 -> (q/k, eb, sh)."""
                dst, w, bias = ((qt, wq_t, bq_t), (kt, wk_t, bk_t))[gi // 4]
                eb, sh = (gi // 2) % 2, gi % 2
                ps = psa.tile([P, SH], f32, tag="acc")
                for sc in range(SH // SC):
                    ssl = bass.ds(sh * SH + sc * SC, SC)
                    psl = bass.ts(sc, SC)
                    for db in range(NDB):
                        nc.tensor.matmul(
                            ps[:, psl],
                            w[:, db, bass.ts(eb, P)],
                            xt[:, db, ssl],
                            start=(db == 0),
                            stop=(db == NDB - 1),
                        )
                # bias-add on ACT for b>0 (idle during proj); DVE for b==0
                # (batch 0's ACT is fully loaded by its own exps)
                if b == 0:
                    nc.vector.tensor_scalar_add(
                        dst[:, eb, bass.ts(sh, SH)], ps[:], bias[:, eb, :]
                    )
                else:
                    nc.scalar.activation(
                        dst[:, eb, bass.ts(sh, SH)], ps[:], IDENT, bias=bias[:, eb, :]
                    )

            def emit_v_group(b, xt, v_all, vg):
                psv = psa.tile([P, VG * D], f32, tag="acc")
                for k in range(VG):
                    tb = vg * VG + k
                    for db in range(NDB):
                        nc.tensor.matmul(
                            psv[:, bass.ts(k, D)],
                            xt[:, db, bass.ts(tb, P)],
                            wv_t[:, db, :],
                            start=(db == 0),
                            stop=(db == NDB - 1),
                        )
                nc.vector.tensor_add(
                    v_all[:, bass.ds(vg * VG, VG), :],
                    psv[:].rearrange("p (g e) -> p g e", g=VG),
                    bvb_t[:].rearrange("p (g e) -> p g e", g=VG),
                )

            def emit_scores_tb(b, t, qt, kt, v_all, tb):
                """scores + exp->fp8 P for one t-block; after the 4th tb in a
                group of 4, emit the denom/reciprocal/V'' DVE ops for the group."""
                for sh in range(NSH):
                    pssc = psa.tile([P, SH], f32, tag="acc")
                    for sc in range(SH // SC):
                        ssl = bass.ds(sh * SH + sc * SC, SC)
                        psl = bass.ts(sc, SC)
                        for eb in range(NDB):
                            nc.tensor.matmul(
                                pssc[:, psl],
                                kt[:, eb, bass.ts(tb, P)],
                                qt[:, eb, ssl],
                                start=(eb == 0),
                                stop=(eb == NDB - 1),
                            )
                    nc.scalar.activation(
                        t["pt"][:, tb, bass.ts(sh, SH)],
                        pssc[:],
                        EXP,
                        bias=shift_t[:],
                        accum_out=t["dnp"][:, tb, sh : sh + 1],
                    )
                if tb % 4 == 3:
                    g0 = tb - 3
                    dn = spool.tile([P, 4], f32, tag="dn")
                    nc.vector.tensor_add(
                        dn[:], t["dnp"][:, bass.ds(g0, 4), 0], t["dnp"][:, bass.ds(g0, 4), 1]
                    )
                    nc.vector.reciprocal(t["rc"][:, bass.ds(g0, 4)], dn[:])
                    for j in range(g0, g0 + 4):
                        # V'' split: t16 = 16*V'', vp = e4m3(V''), vp2 = e4m3(16*(V''-vp))
                        t16 = spool.tile([P, D], f32, tag="t16")
                        nc.vector.tensor_scalar(
                            t16[:],
                            v_all[:, j, :],
                            t["rc"][:, j : j + 1],
                            16.0 * VS,
                            mybir.AluOpType.mult,
                            mybir.AluOpType.mult,
                        )
                        nc.vector.tensor_scalar_mul(t["vp"][:, j, :], t16[:], 1.0 / 16.0)
                        # NOTE: scalar_tensor_tensor with fp8 OUT is broken on HW;
                        # use mul->f32 temp + sub->fp8 instead
                        u16 = spool.tile([P, D], f32, tag="u16")
                        nc.vector.tensor_scalar_mul(u16[:], t["vp"][:, j, :], 16.0)
                        nc.vector.tensor_sub(t["vp2"][:, j, :], t16[:], u16[:])

            def emit_zt_chunk(b, t, zt, ci, lo=False, jr=(0, NTB // 2), pszt=None):
                """One ZT psum sub-chunk (hi: V1 or lo: V2 sum) for (sh, eh, sq).
                jr limits the pair range (split emission keeps the psum group
                open across calls); the zt combine fires when the group closes."""
                sh, eh, sq2 = ci // 4, (ci // 2) % 2, ci % 2
                sq = sh * 2 + sq2
                vsrc = t["vp2"] if lo else t["vp"]
                if pszt is None:
                    pszt = psz.tile([P, SC], f32, tag="z")
                ssl = bass.ts(sq, SC)
                for j in range(*jr):
                    nc.tensor.matmul(
                        pszt[:],
                        vsrc[:, bass.ds(2 * j, 2), bass.ts(eh, P)],
                        t["pt"][:, bass.ds(2 * j, 2), ssl],
                        start=(j == 0),
                        stop=(j == NTB // 2 - 1),
                        perf_mode=DR,
                    )
                if jr[1] < NTB // 2:
                    return pszt
                zsl = zt[:, eh, ssl]
                if not lo:
                    nc.vector.tensor_copy(zsl, pszt[:])
                else:
                    nc.vector.scalar_tensor_tensor(
                        zsl, pszt[:], 1.0 / 16.0, zsl,
                        mybir.AluOpType.mult, mybir.AluOpType.add,
                    )
                return None

            def emit_out_group(b, zt, gi, fan=2):
                """Out-proj psum group (ob, sh): 4 matmuls + DVE bias + DMA."""
                ob, sh = gi // NSH, gi % NSH
                pso = psa.tile([P, SH], f32, tag="acc")
                for sc in range(SH // SC):
                    ssl = bass.ds(sh * SH + sc * SC, SC)
                    psl = bass.ts(sc, SC)
                    for eh in range(NDB):
                        nc.tensor.matmul(
                            pso[:, psl],
                            wo_t[:, eh, bass.ts(ob, P)],
                            zt[:, eh, ssl],
                            start=(eh == 0),
                            stop=(eh == NDB - 1),
                        )
                osb = opool.tile([P, SH], f32, tag="osb")
                if fan == 2:
                    nc.vector.tensor_scalar_add(osb[:], pso[:], bo_t[:, ob, :])
                    dma_eng = nc.sync if gi % 2 == 0 else nc.gpsimd
                    dma_eng.dma_start(
                        out=outT[b, bass.ts(ob, P), bass.ts(sh, SH)], in_=osb[:]
                    )
                else:
                    # final batch: per-512 bias+DMA on sync/scalar queues only
                    # (keep the gpsimd queue empty for a cheap final drain)
                    engs = (nc.sync, nc.scalar)
                    for q in range(2):
                        qsl = bass.ts(q, SH // 2)
                        nc.vector.tensor_scalar_add(
                            osb[:, qsl], pso[:, qsl], bo_t[:, ob, :]
                        )
                        hsl = bass.ds(sh * SH + q * (SH // 2), SH // 2)
                        engs[q].dma_start(
                            out=outT[b, bass.ts(ob, P), hsl], in_=osb[:, qsl]
                        )

            # ======== schedule ========
            xt = xpool.tile([P, NDB, S], f32r, tag="xt")
            qt = xpool.tile([P, NDB, S], f32r, tag="qt")
            kt = xpool.tile([P, NDB, S], f32r, tag="kt")
            v_all = xpool.tile([P, NTB, D], f32, tag="v")
            zt = xpool.tile([P, NDB, S], f32r, tag="zt")

            # interleave item lists: "c<i>h"/"c<i>l" = ZT(b-1) sub-chunk hi/lo,
            # "g<i>" = out(b-1) group
            PROJ_SLOTS = {2: "c0h", 4: "c0l", 6: "c1h", 8: "c1l",
                          10: "c2h", 12: "c2l"}
            SCORE_SLOTS = {2: "c3h", 3: "c3l", 4: "c4h", 5: "g0",
                           6: "c4l", 7: "c5h", 8: "g2", 9: "c5l",
                           10: "c6h", 11: "c6l", 12: "c7h", 13: "c7l",
                           14: "g1", 15: "g3"}

            def emit_item(item, pb, pt_, fan=2):
                if item[0] == "c":
                    emit_zt_chunk(pb, pt_, zt, int(item[1]), lo=item[2] == "l")
                else:
                    emit_out_group(pb, zt, int(item[1]), fan=fan)

            prev = None   # (b-1, tiles)
            for b in range(B):
                cur = batch_tiles()
                if b == 0:
                    emit_xload(b, xt, chunks=4)
                # projections with ZT(b-1) sub-chunks interleaved
                pg = 0
                for gi in range(8):
                    emit_qk_group(b, xt, qt, kt, gi)
                    pg += 1
                    if prev is not None and pg in PROJ_SLOTS:
                        emit_item(PROJ_SLOTS[pg], *prev)
                if prev is not None:
                    for vg in range(NTB // VG):
                        emit_v_group(b, xt, v_all, vg)
                        pg += 1
                        if pg in PROJ_SLOTS:
                            emit_item(PROJ_SLOTS[pg], *prev)
                if prev is not None and b + 1 < B:
                    emit_xload(b + 1, xt)
                for tb in range(NTB):
                    emit_scores_tb(b, cur, qt, kt, v_all, tb)
                    if prev is not None and tb in SCORE_SLOTS:
                        emit_item(SCORE_SLOTS[tb], *prev)
                    elif prev is None and tb % 4 == 0:
                        # batch 0: V-proj groups spread across the phase
                        emit_v_group(b, xt, v_all, tb // 4)
                        if tb // 4 == NTB // VG - 1:
                            # xt(1) load only after the last V-proj read of xt(0)
                            emit_xload(b + 1, xt)
                prev = (b, cur)
            # tail: last batch's ZT + out. The first two hi chunks start with
            # pairs 0..5 (need only vp[0..11], ready before the final exps),
            # closing with pairs 6,7 once the last vp group lands.
            pb, pt_ = prev
            h6 = NTB // 2 - 2
            ps_c0 = emit_zt_chunk(pb, pt_, zt, 0, jr=(0, h6))
            ps_c1 = emit_zt_chunk(pb, pt_, zt, 1, jr=(0, h6))
            emit_zt_chunk(pb, pt_, zt, 0, jr=(h6, NTB // 2), pszt=ps_c0)
            emit_zt_chunk(pb, pt_, zt, 1, jr=(h6, NTB // 2), pszt=ps_c1)
            for item in ("c2h", "c3h", "c0l", "c1l", "c2l", "c3l",
                         "g0", "c4h", "c5h", "g2", "c6h", "c7h", "c4l", "c5l",
                         "c6l", "c7l", "g1", "g3"):
                emit_item(item, pb, pt_, fan=4)

    nc.compile()
    return nc


_NC = None


def _get_nc():
    global _NC
    if _NC is None:
        _NC = _build()
    return _NC


def _make_in_maps(x, Wq, bq, Wk, bk, Wv, bv, Wo, bo):
    x = np.asarray(x, np.float32)
    scale = np.float32(1.0 / np.sqrt(D))
    xT = np.ascontiguousarray(x.transpose(0, 2, 1))
    in_maps = []
    for h in range(H):
        bvh = np.asarray(bv, np.float32)[h]
        m = {
            "xT": xT,
            "wqT": np.ascontiguousarray(np.asarray(Wq, np.float32)[h].T) * scale,
            "wkT": np.ascontiguousarray(np.asarray(Wk, np.float32)[h].T),
            "wvT": np.ascontiguousarray(np.asarray(Wv, np.float32)[h].T),
            "woT": np.ascontiguousarray(np.asarray(Wo, np.float32)[:, h * D : (h + 1) * D].T)
            * (1.0 / VS),
            "bqc": (np.asarray(bq, np.float32)[h] * scale).reshape(D, 1),
            "bkc": np.asarray(bk, np.float32)[h].reshape(D, 1),
            "bvb": np.ascontiguousarray(
                np.broadcast_to(np.tile(bvh, VG), (P, VG * D))
            ),
            "boc": (
                np.asarray(bo, np.float32) if h == 0 else np.zeros(D, np.float32)
            ).reshape(D, 1),
        }
        in_maps.append({k: np.ascontiguousarray(v, np.float32) for k, v in m.items()})
    return in_maps


def kernel(x, Wq, bq, Wk, bk, Wv, bv, Wo, bo, _trace=False, _trace_kwargs=None):
    in_maps = _make_in_maps(x, Wq, bq, Wk, bk, Wv, bv, Wo, bo)
    nc = _get_nc()
    kw = {}
    if _trace:
        kw = dict(trace=True, **(_trace_kwargs or {}))
    br = run_bass_kernel_spmd(nc, in_maps, core_ids=list(range(N_CORES)), **kw)
    acc = np.zeros((B, D, S), np.float32)
    for r in br.results:
        acc += r["outT"]
    out = np.ascontiguousarray(acc.transpose(0, 2, 1))
    if _trace:
        kernel.last_results = br
    return out


# revision 4
# speedup vs baseline: 1.0130x; 1.0023x over previous
"""MultiHeadSelfAttention TRN2 kernel — head-parallel, fp8-DoubleRow ZT path.

Reference semantics (softmax over the QUERY axis):
    Q = x @ Wq[h].T + bq[h]; K likewise; V likewise
    P[s,t] = exp((Q[s]·K[t])/sqrt(D) - shift)        (shift cancels in a)
    denom[t] = sum_s P[s,t]
    Z[s] = sum_t (P[s,t]/denom[t]) V[t]
    out = concat_heads(Z) @ Wo.T + bo

v2 changes vs v1:
  * P written as fp8 e4m3 by ACT exp (shift 6 avoids the 240 clip);
    V'' = V * (VS/denom) split hi/lo into two e4m3 tensors (lo = 16x residual)
    to survive the 2e-2 gate with pre-quantization (ACT accum) denominators.
  * ZT = sum_t V''^T P uses fp8 DoubleRow matmuls (0.55 c/row measured on HW
    vs 1.0 f32r) accumulated directly in PSUM across all 16 t-blocks
    (kills the v1 SBUF Z-accumulation DVE passes).
  * out-proj bias moved to ACT; Z psum->SBUF scale-copy on DVE.
  * batch-level SW pipeline: ZT(b-1)+out(b-1) interleave into scores(b).

Scales: scores f32r exact. P = exp(sc - SHIFT). VS ~ typical denom so V'' ~ O(1).
zt = (1/VS) * psumZ, so out-proj sees true Z.
"""

import numpy as np

import concourse.bass as bass
import concourse.mybir as mybir
import concourse.tile as tile
from concourse import bacc
from concourse.bass_utils import run_bass_kernel_spmd

B, S, D, H = 4, 2048, 256, 8
N_CORES = 8
P = 128          # partitions
NDB = D // P     # 2 d-blocks
NTB = S // P     # 16 t-blocks
SC = 512         # matmul moving chunk
NSC = S // SC    # 4
SH = 1024        # ACT/psum tile width
NSH = S // SH    # 2
VG = 4           # V proj t-blocks per psum group

SHIFT = 6.0      # exp bias shift: P=exp(sc-SHIFT) fits e4m3 (max |sc|~9)
VS = 16.0        # V'' scale ~ typical denom at shift 6; 1/VS folds into woT

f32 = mybir.dt.float32
f32r = mybir.dt.float32r
f8e4 = mybir.dt.float8e4
f8e5 = mybir.dt.float8e5
DR = mybir.MatmulPerfMode.DoubleRow
EXP = mybir.ActivationFunctionType.Exp
IDENT = mybir.ActivationFunctionType.Identity


def _build():
    nc = bacc.Bacc(target_bir_lowering=False)

    xT = nc.dram_tensor("xT", [B, D, S], f32, kind="ExternalInput")
    wqT = nc.dram_tensor("wqT", [D, D], f32, kind="ExternalInput")  # (Wq/sqrt(D)).T
    wkT = nc.dram_tensor("wkT", [D, D], f32, kind="ExternalInput")
    wvT = nc.dram_tensor("wvT", [D, D], f32, kind="ExternalInput")
    woT = nc.dram_tensor("woT", [D, D], f32, kind="ExternalInput")
    bqc = nc.dram_tensor("bqc", [D, 1], f32, kind="ExternalInput")
    bkc = nc.dram_tensor("bkc", [D, 1], f32, kind="ExternalInput")
    bvb = nc.dram_tensor("bvb", [P, VG * D], f32, kind="ExternalInput")
    boc = nc.dram_tensor("boc", [D, 1], f32, kind="ExternalInput")
    outT = nc.dram_tensor("outT", [B, D, S], f32, kind="ExternalOutput")

    with tile.TileContext(nc) as tc:
        with (
            tc.tile_pool(name="const", bufs=1) as cpool,
            tc.tile_pool(name="big", bufs=1) as xpool,
            tc.tile_pool(name="pt", bufs=2) as ppool,
            tc.tile_pool(name="small", bufs=2) as spool,
            tc.tile_pool(name="outp", bufs=2) as opool,
            tc.tile_pool(name="ps_a", bufs=3, space="PSUM") as psa,
            tc.tile_pool(name="ps_z", bufs=2, space="PSUM") as psz,
        ):
            # ---- constants ----
            wq_t = cpool.tile([P, NDB, D], f32r, tag="wq")
            wk_t = cpool.tile([P, NDB, D], f32r, tag="wk")
            wv_t = cpool.tile([P, NDB, D], f32r, tag="wv")
            wo_t = cpool.tile([P, NDB, D], f32r, tag="wo")
            bq_t = cpool.tile([P, NDB, 1], f32, tag="bq")
            bk_t = cpool.tile([P, NDB, 1], f32, tag="bk")
            bo_t = cpool.tile([P, NDB, 1], f32, tag="bo")
            bvb_t = cpool.tile([P, VG * D], f32, tag="bvb")
            # wq + the q/k biases on the scalar queue (gate the first PE/ACT work);
            # everything else behind them on the gpsimd queue
            nc.scalar.dma_start(
                out=wq_t[:], in_=wqT.rearrange("(n p) e -> p n e", p=P).bitcast(f32r)
            )
            for b_t, b_d in ((bq_t, bqc), (bk_t, bkc)):
                nc.scalar.dma_start(
                    out=b_t[:], in_=b_d.rearrange("(n p) o -> p n o", p=P)
                )
            # wo last: not needed until the first out-proj (~60us in)
            for w_t, w_d in ((wk_t, wkT), (wv_t, wvT)):
                nc.gpsimd.dma_start(
                    out=w_t[:], in_=w_d.rearrange("(n p) e -> p n e", p=P).bitcast(f32r)
                )
            nc.gpsimd.dma_start(out=bvb_t[:], in_=bvb[:])
            nc.gpsimd.dma_start(
                out=wo_t[:], in_=woT.rearrange("(n p) e -> p n e", p=P).bitcast(f32r)
            )
            nc.gpsimd.dma_start(
                out=bo_t[:], in_=boc.rearrange("(n p) o -> p n o", p=P)
            )
            shift_t = cpool.tile([P, 1], f32, tag="shiftc")
            nc.vector.memset(shift_t[:], -SHIFT)

            # ---- persistent tiles (single-generation; Tile tracks hazards) ----
            def batch_tiles():
                return dict(
                    pt=ppool.tile([P, NTB, S], f8e4, tag="pt", name="pt"),
                    vp=ppool.tile([P, NTB, D], f8e4, tag="vp", name="vp"),
                    vp2=ppool.tile([P, NTB, D], f8e4, tag="vp2", name="vp2"),
                    dnp=spool.tile([P, NTB, NSH], f32, tag="dnp", name="dnp"),
                    rc=spool.tile([P, NTB], f32, tag="rc", name="rc"),
                )

            # ======== emit helpers (b = batch idx) ========
            def emit_xload(b, xt, chunks=2):
                xT_r = xT[b].rearrange("(n p) s -> p n s", p=P).bitcast(f32r)
                w = S // chunks
                for c in range(chunks):
                    nc.sync.dma_start(
                        out=xt[:, :, bass.ds(c * w, w)], in_=xT_r[:, :, bass.ds(c * w, w)]
                    )

            def emit_qk_group(b, xt, qt, kt, gi):
                """One Q/K projection psum group: gi in 0..7 -> (q/k, eb, sh)."""
                dst, w, bias = ((qt, wq_t, bq_t), (kt, wk_t, bk_t))[gi // 4]
                eb, sh = (gi // 2) % 2, gi % 2
                ps = psa.tile([P, SH], f32, tag="acc")
                for sc in range(SH // SC):
                    ssl = bass.ds(sh * SH + sc * SC, SC)
                    psl = bass.ts(sc, SC)
                    for db in range(NDB):
                        nc.tensor.matmul(
                            ps[:, psl],
                            w[:, db, bass.ts(eb, P)],
                            xt[:, db, ssl],
                            start=(db == 0),
                            stop=(db == NDB - 1),
                        )
                # bias-add on ACT for b>0 (idle during proj); DVE for b==0
                # (batch 0's ACT is fully loaded by its own exps)
                if b == 0:
                    nc.vector.tensor_scalar_add(
                        dst[:, eb, bass.ts(sh, SH)], ps[:], bias[:, eb, :]
                    )
                else:
                    nc.scalar.activation(
                        dst[:, eb, bass.ts(sh, SH)], ps[:], IDENT, bias=bias[:, eb, :]
                    )

            def emit_v_group(b, xt, v_all, vg):
                psv = psa.tile([P, VG * D], f32, tag="acc")
                for k in range(VG):
                    tb = vg * VG + k
                    for db in range(NDB):
                        nc.tensor.matmul(
                            psv[:, bass.ts(k, D)],
                            xt[:, db, bass.ts(tb, P)],
                            wv_t[:, db, :],
                            start=(db == 0),
                            stop=(db == NDB - 1),
                        )
                nc.vector.tensor_add(
                    v_all[:, bass.ds(vg * VG, VG), :],
                    psv[:].rearrange("p (g e) -> p g e", g=VG),
                    bvb_t[:].rearrange("p (g e) -> p g e", g=VG),
                )

            def emit_scores_tb(b, t, qt, kt, v_all, tb):
                """scores + exp->fp8 P for one t-block; after the 4th tb in a
                group of 4, emit the denom/reciprocal/V'' DVE ops for the group."""
                for sh in range(NSH):
                    pssc = psa.tile([P, SH], f32, tag="acc")
                    for sc in range(SH // SC):
                        ssl = bass.ds(sh * SH + sc * SC, SC)
                        psl = bass.ts(sc, SC)
                        for eb in range(NDB):
                            nc.tensor.matmul(
                                pssc[:, psl],
                                kt[:, eb, bass.ts(tb, P)],
                                qt[:, eb, ssl],
                                start=(eb == 0),
                                stop=(eb == NDB - 1),
                            )
                    nc.scalar.activation(
                        t["pt"][:, tb, bass.ts(sh, SH)],
                        pssc[:],
                        EXP,
                        bias=shift_t[:],
                        accum_out=t["dnp"][:, tb, sh : sh + 1],
                    )
                if tb % 4 == 3:
                    g0 = tb - 3
                    dn = spool.tile([P, 4], f32, tag="dn")
                    nc.vector.tensor_add(
                        dn[:], t["dnp"][:, bass.ds(g0, 4), 0], t["dnp"][:, bass.ds(g0, 4), 1]
                    )
                    nc.vector.reciprocal(t["rc"][:, bass.ds(g0, 4)], dn[:])
                    for j in range(g0, g0 + 4):
                        # V'' split: vp = e4m3(V''), vp2 = e4m3(V'' - vp) — the lo
                        # residual is NOT pre-scaled, so ZT can accumulate both
                        # halves in one psum group (no /16 combine needed)
                        tV = spool.tile([P, D], f32, tag="t16")
                        nc.vector.tensor_scalar(
                            tV[:],
                            v_all[:, j, :],
                            t["rc"][:, j : j + 1],
                            VS,
                            mybir.AluOpType.mult,
                            mybir.AluOpType.mult,
                        )
                        nc.vector.tensor_copy(t["vp"][:, j, :], tV[:])
                        # NOTE: scalar_tensor_tensor with fp8 OUT is broken on HW;
                        # read vp back to f32, then sub -> fp8
                        u16 = spool.tile([P, D], f32, tag="u16")
                        nc.vector.tensor_copy(u16[:], t["vp"][:, j, :])
                        nc.vector.tensor_sub(t["vp2"][:, j, :], tV[:], u16[:])

            def emit_zt_chunk(b, t, zt, ci, part=None, pszt=None):
                """One ZT psum chunk for (sh, eh, sq): 16 DR matmuls (8 vp pairs
                + 8 vp2 pairs) in ONE accumulation group, then a single copy.
                part=0 emits the first 6 pairs of each half (ready before the
                final exps); part=1 closes with the last 2 pairs + copy."""
                sh, eh, sq2 = ci // 4, (ci // 2) % 2, ci % 2
                sq = sh * 2 + sq2
                if pszt is None:
                    pszt = psz.tile([P, SC], f32, tag="z")
                ssl = bass.ts(sq, SC)
                NJ = NTB // 2
                jrs = {None: range(2 * NJ), 0: list(range(6)) + list(range(NJ, NJ + 6)),
                       1: [6, 7, NJ + 6, NJ + 7]}[part]
                for idx in jrs:
                    vsrc, j = (t["vp"], idx) if idx < NJ else (t["vp2"], idx - NJ)
                    nc.tensor.matmul(
                        pszt[:],
                        vsrc[:, bass.ds(2 * j, 2), bass.ts(eh, P)],
                        t["pt"][:, bass.ds(2 * j, 2), ssl],
                        start=(idx == 0),
                        stop=(idx == 2 * NJ - 1),
                        perf_mode=DR,
                    )
                if part == 0:
                    return pszt
                nc.vector.tensor_copy(zt[:, eh, ssl], pszt[:])
                return None

            def emit_out_group(b, zt, gi, fan=2):
                """Out-proj psum group (ob, sh): 4 matmuls + DVE bias + DMA."""
                ob, sh = gi // NSH, gi % NSH
                pso = psa.tile([P, SH], f32, tag="acc")
                for sc in range(SH // SC):
                    ssl = bass.ds(sh * SH + sc * SC, SC)
                    psl = bass.ts(sc, SC)
                    for eh in range(NDB):
                        nc.tensor.matmul(
                            pso[:, psl],
                            wo_t[:, eh, bass.ts(ob, P)],
                            zt[:, eh, ssl],
                            start=(eh == 0),
                            stop=(eh == NDB - 1),
                        )
                osb = opool.tile([P, SH], f32, tag="osb")
                if fan == 2:
                    nc.vector.tensor_scalar_add(osb[:], pso[:], bo_t[:, ob, :])
                    dma_eng = nc.sync if gi % 2 == 0 else nc.gpsimd
                    dma_eng.dma_start(
                        out=outT[b, bass.ts(ob, P), bass.ts(sh, SH)], in_=osb[:]
                    )
                else:
                    # final batch: per-512 bias+DMA on sync/scalar queues only
                    # (keep the gpsimd queue empty for a cheap final drain);
                    # the very last groups split bias across DVE/ACT so the two
                    # halves drain in parallel after the final matmuls
                    engs = (nc.sync, nc.scalar)
                    for q in range(2):
                        qsl = bass.ts(q, SH // 2)
                        if gi >= 2 and q == 1:
                            nc.scalar.activation(
                                osb[:, qsl], pso[:, qsl], IDENT, bias=bo_t[:, ob, :]
                            )
                        else:
                            nc.vector.tensor_scalar_add(
                                osb[:, qsl], pso[:, qsl], bo_t[:, ob, :]
                            )
                        hsl = bass.ds(sh * SH + q * (SH // 2), SH // 2)
                        engs[q].dma_start(
                            out=outT[b, bass.ts(ob, P), hsl], in_=osb[:, qsl]
                        )

            # ======== schedule ========
            xt = xpool.tile([P, NDB, S], f32r, tag="xt")
            qt = xpool.tile([P, NDB, S], f32r, tag="qt")
            kt = xpool.tile([P, NDB, S], f32r, tag="kt")
            v_all = xpool.tile([P, NTB, D], f32, tag="v")
            zt = xpool.tile([P, NDB, S], f32r, tag="zt")

            # interleave item lists: "c<i>" = ZT(b-1) chunk, "g<i>" = out(b-1) group
            PROJ_SLOTS = {4: "c0", 8: "c1", 12: "c2"}
            SCORE_SLOTS = {2: "c3", 4: "c4", 5: "g0", 7: "c5", 8: "g2",
                           10: "c6", 12: "c7", 14: "g1", 15: "g3"}

            def emit_item(item, pb, pt_, fan=2):
                if item[0] == "c":
                    emit_zt_chunk(pb, pt_, zt, int(item[1]))
                else:
                    emit_out_group(pb, zt, int(item[1]), fan=fan)

            prev = None   # (b-1, tiles)
            for b in range(B):
                cur = batch_tiles()
                if b == 0:
                    emit_xload(b, xt, chunks=4)
                # projections with ZT(b-1) sub-chunks interleaved
                pg = 0
                for gi in range(8):
                    emit_qk_group(b, xt, qt, kt, gi)
                    pg += 1
                    if prev is not None and pg in PROJ_SLOTS:
                        emit_item(PROJ_SLOTS[pg], *prev)
                if prev is not None:
                    for vg in range(NTB // VG):
                        emit_v_group(b, xt, v_all, vg)
                        pg += 1
                        if pg in PROJ_SLOTS:
                            emit_item(PROJ_SLOTS[pg], *prev)
                if prev is not None and b + 1 < B:
                    emit_xload(b + 1, xt)
                for tb in range(NTB):
                    emit_scores_tb(b, cur, qt, kt, v_all, tb)
                    if prev is not None and tb in SCORE_SLOTS:
                        emit_item(SCORE_SLOTS[tb], *prev)
                    elif prev is None and tb % 4 == 0:
                        # batch 0: V-proj groups spread across the phase
                        emit_v_group(b, xt, v_all, tb // 4)
                        if tb // 4 == NTB // VG - 1:
                            # xt(1) load only after the last V-proj read of xt(0)
                            emit_xload(b + 1, xt)
                prev = (b, cur)
            # tail: last batch's ZT + out. The first two chunks emit their
            # first 12 matmuls early (need only vp/vp2[0..11], ready before the
            # final exps), closing once the last vp group lands.
            pb, pt_ = prev
            ps_c0 = emit_zt_chunk(pb, pt_, zt, 0, part=0)
            ps_c1 = emit_zt_chunk(pb, pt_, zt, 1, part=0)
            emit_zt_chunk(pb, pt_, zt, 0, part=1, pszt=ps_c0)
            emit_zt_chunk(pb, pt_, zt, 1, part=1, pszt=ps_c1)
            for item in ("c2", "c3", "g0", "c4", "g2", "c5", "c6", "c7",
                         "g1", "g3"):
                emit_item(item, pb, pt_, fan=4)

    nc.compile()
    return nc


_NC = None


def _get_nc():
    global _NC
    if _NC is None:
        _NC = _build()
    return _NC


def _make_in_maps(x, Wq, bq, Wk, bk, Wv, bv, Wo, bo):
    x = np.asarray(x, np.float32)
    scale = np.float32(1.0 / np.sqrt(D))
    xT = np.ascontiguousarray(x.transpose(0, 2, 1))
    in_maps = []
    for h in range(H):
        bvh = np.asarray(bv, np.float32)[h]
        m = {
            "xT": xT,
            "wqT": np.ascontiguousarray(np.asarray(Wq, np.float32)[h].T) * scale,
            "wkT": np.ascontiguousarray(np.asarray(Wk, np.float32)[h].T),
            "wvT": np.ascontiguousarray(np.asarray(Wv, np.float32)[h].T),
            "woT": np.ascontiguousarray(np.asarray(Wo, np.float32)[:, h * D : (h + 1) * D].T)
            * (1.0 / VS),
            "bqc": (np.asarray(bq, np.float32)[h] * scale).reshape(D, 1),
            "bkc": np.asarray(bk, np.float32)[h].reshape(D, 1),
            "bvb": np.ascontiguousarray(
                np.broadcast_to(np.tile(bvh, VG), (P, VG * D))
            ),
            "boc": (
                np.asarray(bo, np.float32) if h == 0 else np.zeros(D, np.float32)
            ).reshape(D, 1),
        }
        in_maps.append({k: np.ascontiguousarray(v, np.float32) for k, v in m.items()})
    return in_maps


def kernel(x, Wq, bq, Wk, bk, Wv, bv, Wo, bo, _trace=False, _trace_kwargs=None):
    in_maps = _make_in_maps(x, Wq, bq, Wk, bk, Wv, bv, Wo, bo)
    nc = _get_nc()
    kw = {}
    if _trace:
        kw = dict(trace=True, **(_trace_kwargs or {}))
    br = run_bass_kernel_spmd(nc, in_maps, core_ids=list(range(N_CORES)), **kw)
    acc = np.zeros((B, D, S), np.float32)
    for r in br.results:
        acc += r["outT"]
    out = np.ascontiguousarray(acc.transpose(0, 2, 1))
    if _trace:
        kernel.last_results = br
    return out
